# revision 1
# baseline (speedup 1.0000x reference)
"""Causal attention (B=4, S=2048, D=1024, single head) on 8 TRN2 NeuronCores.

Sharding: data-parallel over batch (4 pairs of cores); within each pair
the K/V context is split by interleaved 128-row chunks (core parity p
owns global k-chunks {2j+p}).  Each core projects K/V for its own 1024
context rows and Q for its own 1024 rows; the pair exchanges Q halves
with a 2-core AllGather so both cores hold Q for all 2048 rows in a
canonical "gathered" column order ([all even 128-blocks | all odd
128-blocks] — rank-indexed, hence identical on both cores).  Each core
then computes its causal score blocks against its own context and
produces *unnormalized* partial attention output plus the per-row
partial softmax denominator.  The host adds the two partials of each
pair and normalizes.

The SPMD program is identical across cores; all parity-dependent causal
structure lives in input data (per-core column-permuted x, per-core mask
tiles).  All matmuls run in bf16 (fp32 PSUM accumulation); inputs are
pre-cast on the host.

v2 schedule notes:
- Inputs arrive via 8 large strided DMAs spread over 4 engine queues
  (sync/scalar/gpsimd/vector) instead of ~49 per-chunk DMAs: each
  dma_start costs ~0.7us of issue time on its queue, which starved the
  early projection matmuls of weight slices in v1.
- Q projection runs o-chunks {0,1} for all i first (they arrive in the
  first wq DMA), then o-chunks {2..7}, so the PE never waits on the
  second wq piece.
- A short burst of garbage warm-up matmuls issues before any real work
  to bring the PE out of its cold HAM clock state during the DMA fill.
- Partial outputs are written in bf16 (host accumulates in fp32), which
  halves the output DMA bytes and the end-of-kernel DMA tail.
"""

import sys

if "/opt/trn_rl_repo" not in sys.path:
    sys.path.insert(0, "/opt/trn_rl_repo")

import ml_dtypes
import numpy as np

import concourse.bacc as bacc
import concourse.tile as tile
from concourse import mybir
from concourse.bass_utils import run_bass_kernel_spmd

# bass_utils imports antenv.axon_hooks when tracing is requested; the image's
# antenv lacks that module, so provide a no-op fallback rather than crashing.
try:
    import antenv.axon_hooks  # noqa: F401
except ImportError:
    import types as _types

    _ah = _types.ModuleType("antenv.axon_hooks")
    _ah._hook = None
    _ah.set_axon_ntff_profile_hook = lambda h: setattr(_ah, "_hook", h)
    _ah.get_axon_ntff_profile_hook = lambda: _ah._hook
    sys.modules["antenv.axon_hooks"] = _ah

B, S, D = 4, 2048, 1024
NB = S // 128          # 16 q-blocks of 128 per batch
NT = S // 512          # 4 q-tiles of 512
IC = D // 128          # 8 contraction chunks
OC = D // 128          # 8 output-dim chunks
LC = 8                 # local k-chunks per core (S/2/128)
NMSK = 16              # mask tiles: 4 per q-tile
SCALE = 1.0 / np.sqrt(D)  # 0.03125
NJ_TILE = [4, 8, 4, 8]  # local k-chunks needed per gathered q-tile
NWARM = 4              # PE warm-up matmuls

BF16 = mybir.dt.bfloat16
F32 = mybir.dt.float32

_module_cache = None
last_results = None  # BassKernelResults of the most recent run (for test harness)


def _masked_js(tt):
    """Local chunk indices whose score blocks need a mask for q-tile tt."""
    return range(4) if tt in (0, 2) else range(4, 8)


def _build_module():
    nc = bacc.Bacc("TRN2", target_bir_lowering=False, debug=False, num_devices=8)
    # xT is half-major: [h, i, p, c] flattened — one contiguous slab per
    # sequence half so each half is a single contiguous-read DMA.
    # All inputs are packed partition-major on the host so every input DMA
    # moves multi-KB contiguous runs on both the DRAM and SBUF side (small
    # strided lines are descriptor-overhead-bound at ~70-120 GB/s).
    xT = nc.dram_tensor("xT", [2, 128, IC, 512], BF16, kind="ExternalInput").ap()
    wq4 = nc.dram_tensor("wq4", [4, 128, IC, 256], BF16, kind="ExternalInput").ap()
    wkT = nc.dram_tensor("wkT", [128, IC, 1024], BF16, kind="ExternalInput").ap()
    wvT = nc.dram_tensor("wvT", [128, IC, 1024], BF16, kind="ExternalInput").ap()
    msk = nc.dram_tensor("msk", [128, NMSK, 512], BF16, kind="ExternalInput").ap()
    out_p = nc.dram_tensor("out_p", [S, D], BF16, kind="ExternalOutput").ap()
    rs_out = nc.dram_tensor("rs_out", [1, S], F32, kind="ExternalOutput").ap()

    with tile.TileContext(nc) as tc:
        with (
            tc.tile_pool(name="wp", bufs=1) as wp,
            tc.tile_pool(name="xp", bufs=1) as xp,
            tc.tile_pool(name="kqv", bufs=1) as kqv,
            tc.tile_pool(name="mp", bufs=1) as mp,
            tc.tile_pool(name="ptp", bufs=2) as ptp,
            tc.tile_pool(name="stg", bufs=4) as stg,
            tc.tile_pool(name="qsg", bufs=2) as qsg,
            tc.tile_pool(name="dr", bufs=1, space="DRAM") as dr,
        ):
            # ---- PE warm-up: garbage matmuls on zeroed tiles, issued
            #      before any input-dependent work so the PE leaves its
            #      cold clock state while input DMAs are in flight ----
            warm_w = mp.tile([128, 128], BF16, tag="warmw", name="warmw")
            warm_x = mp.tile([128, 512], BF16, tag="warmx", name="warmx")
            nc.gpsimd.memset(warm_w, 0.0)
            nc.gpsimd.memset(warm_x, 0.0)
            with tc.tile_pool(name="psw", bufs=1, space="PSUM") as psw:
                wpp = psw.tile([128, 512], F32, tag="warm", bufs=1, name="warmp")
                for _ in range(NWARM):
                    nc.tensor.matmul(wpp, lhsT=warm_w, rhs=warm_x, start=True, stop=True)

            # consolidated input tiles, laid out so each input DMA writes a
            # contiguous per-partition run (kilobyte-scale DMA lines):
            # xt h-major, wq o-group-major
            xt_all = xp.tile([128, 2, IC, 512], BF16, tag="xt", name="xt")
            wq_all = wp.tile([128, 4, IC, 256], BF16, tag="wq", name="wq")
            wk_all = wp.tile([128, IC, 1024], BF16, tag="wk", name="wk")
            wv_all = wp.tile([128, IC, 1024], BF16, tag="wv", name="wv")

            # Queues are FIFO and HBM saturates when several large DMAs run
            # concurrently, so order the pieces by when the PE needs them:
            # sync: x h0 (two pieces) -> x h1 -> wk; scalar: 4 wq o-group
            # slabs -> wv -> masks.  gpsimd carries no input DMAs (it runs
            # the collectives and output DMAs).
            # x half 0 in two pieces and the first wq slab in two, so the
            # first projection matmuls start as soon as ~0.25MB has landed
            nc.sync.dma_start(xt_all[:, 0, 0:4, :], xT[0, :, 0:4, :])
            nc.sync.dma_start(xt_all[:, 0, 4:8, :], xT[0, :, 4:8, :])
            nc.sync.dma_start(xt_all[:, 1, :, :], xT[1])
            for g in range(4):
                nc.scalar.dma_start(wq_all[:, g, :, :], wq4[g])
            nc.sync.dma_start(wk_all, wkT)
            nc.scalar.dma_start(wv_all, wvT)
            # masks are not needed until the attention phase
            mask_all = mp.tile([128, NMSK, 512], BF16, tag="masks", name="masks")
            nc.scalar.dma_start(mask_all, msk)
            ones_sb = mp.tile([128, 1], BF16, tag="ones", name="ones")
            nc.any.memset(ones_sb, 1.0)

            kt_sb = [kqv.tile([128, S // 2], BF16, tag=f"kt{o}", name=f"kt{o}") for o in range(OC)]
            qt_all = kqv.tile([128, OC, S], BF16, tag="qt", name="qt")
            vn_sb = [kqv.tile([128, D], BF16, tag=f"vn{j}", name=f"vn{j}") for j in range(LC)]

            # DRAM bounce buffers for the pairwise Q-half exchange (split in
            # two st-pieces so the exchange pipelines with the projections)
            qhalf = [dr.tile([128, OC * 512], BF16, name=f"qhalf{st}") for st in range(2)]
            qfull = [dr.tile([2 * 128, OC * 512], BF16, name=f"qfull{st}") for st in range(2)]

            def xs(i, h):
                return xt_all[:, h, i, :]

            def proj_iouter(ps1, lhs_slices, rhs_slices, dsts, pname, o_order=None):
                o_order = o_order or [range(len(dsts))]
                pps = [
                    ps1.tile([128, 512], F32, tag=f"proj8_{o}", bufs=1, name=f"{pname}{o}")
                    for o in range(len(dsts))
                ]
                for orng in o_order:
                    for i in range(IC):
                        for o in orng:
                            nc.tensor.matmul(
                                pps[o],
                                lhsT=lhs_slices(i, o),
                                rhs=rhs_slices(i, o),
                                start=(i == 0),
                                stop=(i == IC - 1),
                            )
                for o, dst in enumerate(dsts):
                    dst(pps[o])

            def copy_to(dst):
                # phase-1 copies go to DVE: the Scalar engine services its
                # own queue early on, and a copy stuck behind other issues
                # delays PSUM-bank recycling
                return lambda pp: nc.vector.tensor_copy(dst, pp)

            def q_own_phase(ps1, st, o_order=None):
                """Project this core's own Q half (local even blocks of x),
                stage to SBUF, then one batched DMA into qhalf[st]."""
                qs = qsg.tile([128, OC, 512], BF16, tag="qsg", name="qsg")
                proj_iouter(
                    ps1,
                    lambda i, o: wq_all[:, o // 2, i, 128 * (o % 2) : 128 * (o % 2 + 1)],
                    lambda i, o: xs(i, st),
                    [copy_to(qs[:, o, :]) for o in range(OC)],
                    f"pq{st}",
                    o_order=o_order,
                )
                nc.sync.dma_start(
                    qhalf[st].rearrange("p (o c) -> p o c", o=OC), qs
                )

            def q_exchange(st):
                nc.gpsimd.collective_compute(
                    kind="AllGather",
                    op=mybir.AluOpType.bypass,
                    replica_groups=[[0, 1], [2, 3], [4, 5], [6, 7]],
                    ins=[qhalf[st]],
                    outs=[qfull[st]],
                )
                for r in range(2):
                    eng = nc.sync if r == 0 else nc.scalar
                    eng.dma_start(
                        qt_all[:, :, 1024 * r + 512 * st : 1024 * r + 512 * (st + 1)],
                        qfull[st][128 * r : 128 * (r + 1), :].rearrange(
                            "p (o c) -> p o c", o=OC
                        ),
                    )

            with tc.tile_pool(name="ps1", bufs=1, space="PSUM") as ps1:
                # o-chunk pairs arrive one wq slab at a time; run each pair
                # for all i so the PE tracks the wq DMA arrival order
                q_own_phase(
                    ps1, 0, o_order=[range(2 * g, 2 * g + 2) for g in range(4)]
                )
                q_exchange(0)
                q_own_phase(ps1, 1)
                q_exchange(1)
                proj_iouter(
                    ps1,
                    lambda i, o: wk_all[:, i, 128 * o : 128 * (o + 1)],
                    lambda i, o: xs(i, 0),
                    [copy_to(kt_sb[o][:, 0:512]) for o in range(OC)],
                    "pk0",
                )
                proj_iouter(
                    ps1,
                    lambda i, o: wk_all[:, i, 128 * o : 128 * (o + 1)],
                    lambda i, o: xs(i, 1),
                    [copy_to(kt_sb[o][:, 512:1024]) for o in range(OC)],
                    "pk1",
                )
                # V projection for chunks j<4 (all that attention tiles 0 and
                # 2 need); chunks j>=4 are projected later, between attention
                # tiles 2 and 1, to cover the latency of the second Q exchange
                proj_iouter(
                    ps1,
                    lambda i, c: xt_all[:, 0, i, 128 * (c // 2) : 128 * (c // 2 + 1)],
                    lambda i, c: wv_all[:, i, 512 * (c % 2) : 512 * (c % 2 + 1)],
                    [
                        (lambda dst: lambda pp: nc.any.tensor_copy(dst, pp))(
                            vn_sb[c // 2][:, 512 * (c % 2) : 512 * (c % 2 + 1)]
                        )
                        for c in range(8)
                    ],
                    "pva",
                )

            # ---- phase 2: attention over gathered q-tiles; tiles 0 and 2
            #      only depend on the first exchange piece ----
            rs_sb = mp.tile([1, S], F32, tag="rs", name="rs")
            with tc.tile_pool(name="ps2", bufs=2, space="PSUM") as ps:

                def vn_late():
                    for c in range(8):
                        j, ot = 4 + c // 2, c % 2
                        pp = ps.tile([128, 512], F32, tag="score", bufs=4, name="pvb")
                        for i in range(IC):
                            nc.tensor.matmul(
                                pp,
                                lhsT=xt_all[:, 1, i, 128 * (j - 4) : 128 * (j - 3)],
                                rhs=wv_all[:, i, 512 * ot : 512 * (ot + 1)],
                                start=(i == 0),
                                stop=(i == IC - 1),
                            )
                        if c % 2 == 0:
                            nc.vector.tensor_copy(
                                vn_sb[j][:, 512 * ot : 512 * (ot + 1)], pp
                            )
                        else:
                            nc.scalar.copy(
                                vn_sb[j][:, 512 * ot : 512 * (ot + 1)], pp
                            )

                def attention_tile(tt, out_eng):
                    nj = NJ_TILE[tt]
                    masked = set(_masked_js(tt))
                    pt_tiles = []
                    offs = []
                    for j in range(nj):
                        # in a masked (diagonal-region) block, the first
                        # 128*(j%4) gathered q-columns are fully masked out —
                        # skip computing them entirely
                        off = 128 * (j % 4) if j in masked else 0
                        offs.append(off)
                        sp = ps.tile([128, 512], F32, tag="score", bufs=4, name="score")
                        for o in range(OC):
                            nc.tensor.matmul(
                                sp[:, off:512],
                                lhsT=kt_sb[o][:, 128 * j : 128 * (j + 1)],
                                rhs=qt_all[:, o, 512 * tt + off : 512 * (tt + 1)],
                                start=(o == 0),
                                stop=(o == OC - 1),
                            )
                        pt = ptp.tile([128, 512], BF16, tag=f"pt{j}", name=f"pt{j}")
                        nc.scalar.activation(
                            pt[:, off:512],
                            sp[:, off:512],
                            mybir.ActivationFunctionType.Exp,
                            scale=SCALE,
                        )
                        if j in masked:
                            m = 4 * tt + (j % 4)
                            nc.vector.tensor_mul(
                                pt[:, off:512], pt[:, off:512], mask_all[:, m, off:512]
                            )
                        pt_tiles.append(pt)

                    # partial softmax denominators: ones^T @ pt accumulated over j
                    rsp = ps.tile([1, 512], F32, tag="rs", bufs=1, name="rsp")
                    for j in range(nj):
                        nc.tensor.matmul(
                            rsp[:, offs[j] : 512],
                            lhsT=ones_sb,
                            rhs=pt_tiles[j][:, offs[j] : 512],
                            start=(j == 0),
                            stop=(j == nj - 1),
                        )
                    nc.vector.tensor_copy(rs_sb[:, 512 * tt : 512 * (tt + 1)], rsp)
                    out_eng.dma_start(
                        rs_out[:, 512 * tt : 512 * (tt + 1)],
                        rs_sb[:, 512 * tt : 512 * (tt + 1)],
                    )

                    for qq in (3, 2, 1, 0):
                        qbg = 4 * tt + qq        # gathered q-block index
                        njs = (qbg % 8) + 1      # causal chunk count in gathered order
                        ost = stg.tile([128, D], BF16, tag="ost", name="ost")
                        for ot in range(2):
                            apsum = ps.tile(
                                [128, 512], F32, tag="attn", bufs=3, name="attn"
                            )
                            for j in range(njs):
                                nc.tensor.matmul(
                                    apsum,
                                    lhsT=pt_tiles[j][:, 128 * qq : 128 * (qq + 1)],
                                    rhs=vn_sb[j][:, 512 * ot : 512 * (ot + 1)],
                                    start=(j == 0),
                                    stop=(j == njs - 1),
                                )
                            # alternate copy engines and DMA each half as soon
                            # as its copy lands — shortens the end-of-kernel
                            # copy+DMA tail
                            if ot == 0:
                                nc.vector.tensor_copy(ost[:, 0:512], apsum)
                            else:
                                nc.scalar.copy(ost[:, 512:1024], apsum)
                            out_eng.dma_start(
                                out_p[128 * qbg : 128 * (qbg + 1), 512 * ot : 512 * (ot + 1)],
                                ost[:, 512 * ot : 512 * (ot + 1)],
                            )
                attention_tile(0, nc.gpsimd)
                attention_tile(2, nc.sync)
                vn_late()
                attention_tile(1, nc.gpsimd)
                attention_tile(3, nc.sync)

    nc.compile()
    return nc


def _get_module():
    global _module_cache
    if _module_cache is None:
        _module_cache = _build_module()
    return _module_cache


def _gathered_q(p):
    """Global q index for gathered position p (vectorized)."""
    p = np.asarray(p)
    blk = p // 128
    even = blk < 8
    gb = np.where(even, 2 * blk, 2 * (blk - 8) + 1)
    return 128 * gb + p % 128


def _host_masks(par: int) -> np.ndarray:
    """[NMSK*128, 512] bf16 causal masks in gathered q order."""
    out = np.zeros((NMSK * 128, 512), dtype=np.float32)
    k = np.arange(128)[:, None]
    ql = np.arange(512)[None, :]
    for tt in range(NT):
        for idx, j in enumerate(_masked_js(tt)):
            m = 4 * tt + idx
            g = 2 * j + par  # global k-chunk of local chunk j
            q_global = _gathered_q(512 * tt + ql)
            out[128 * m : 128 * (m + 1), :] = (q_global >= 128 * g + k).astype(
                np.float32
            )
    return out.astype(ml_dtypes.bfloat16)


def kernel(x, Wq, Wk, Wv, _trace=False):
    global last_results
    nc = _get_module()

    bf = ml_dtypes.bfloat16

    wqT_f = Wq.T.astype(bf)
    # pack wq partition-major as 4 o-group slabs: [g, p, i, 256]
    wq4 = np.ascontiguousarray(
        wqT_f.reshape(IC, 128, 4, 256).transpose(2, 1, 0, 3)
    )
    # wk/wv partition-major: [p, i, 1024]
    wkT = np.ascontiguousarray(Wk.T.astype(bf).reshape(IC, 128, D).transpose(1, 0, 2))
    wvT = np.ascontiguousarray(Wv.T.astype(bf).reshape(IC, 128, D).transpose(1, 0, 2))
    masks = [
        np.ascontiguousarray(
            _host_masks(par).reshape(NMSK, 128, 512).transpose(1, 0, 2)
        )
        for par in range(2)
    ]

    # per-parity column selection: core owns global k-chunks {2j+par}
    own_cols = [
        (128 * (2 * np.arange(LC)[:, None] + par) + np.arange(128)[None, :]).reshape(-1)
        for par in range(2)
    ]

    in_maps = []
    for c in range(8):
        b, par = c // 2, c % 2
        xTb = x[b].T[:, own_cols[par]].astype(bf)  # [D, S//2]
        # pack partition-major: [h, p, i, c]
        xpk = np.ascontiguousarray(
            xTb.reshape(IC, 128, 2, 512).transpose(2, 1, 0, 3)
        )
        in_maps.append(
            {
                "xT": xpk,
                "wq4": wq4,
                "wkT": wkT,
                "wvT": wvT,
                "msk": masks[par],
            }
        )

    kwargs = {}
    if _trace:
        kwargs["trace"] = True
    res = run_bass_kernel_spmd(nc, in_maps, core_ids=list(range(8)), **kwargs)
    last_results = res

    # rows come back in gathered order; gath_row[q] = gathered position of q
    gath_row = np.empty(S, dtype=np.int64)
    gath_row[_gathered_q(np.arange(S))] = np.arange(S)

    out = np.empty((B, S, D), dtype=np.float32)
    for b in range(B):
        rA = res.results[2 * b]
        rB = res.results[2 * b + 1]
        num = rA["out_p"].astype(np.float32) + rB["out_p"].astype(np.float32)
        den = rA["rs_out"][0] + rB["rs_out"][0]
        out[b] = (num / den[:, None])[gath_row]
    return out

